# revision 6
# baseline (speedup 1.0000x reference)
"""Trainium2 Bass kernel for nn_Decoder_57758720196948.

Bahdanau-attention decoder step:
  score   = tanh(encoderOut @ W1 + b1 + (hidden @ W2 + b2)[:,None,:])
  attenW  = softmax(score @ v + bv, axis=L)          (bv drops out of softmax)
  context = sum(attenW * encoderOut, axis=L)
  x       = [context ; emb[X]]
  GRU with h_prev = 0  =>  hm = gru_b[1], gru_rk unused,
      z = sigmoid(xz+hz), r = sigmoid(xr+hr), cand = tanh(xh + r*hh),
      h' = (1-z)*cand
  o1      = h' @ fc1_w + fc1_b
  logits  = o1 @ fc2_w + fc2_b

Sharding: launch 1 is data-parallel over batch (32 rows/core); launch 2 is
tensor-parallel over the vocab axis of fc2 (6656 padded cols/core), with the
gather of logits done host-side. Matmuls run in bf16 with fp32 PSUM
accumulation; softmax/GRU elementwise math in fp32.
"""
import numpy as np
import ml_dtypes

import concourse.bass as bass
import concourse.tile as tile
from concourse import bacc, mybir
from concourse.bass_utils import run_bass_kernel_spmd
from concourse.masks import make_identity

bf16 = mybir.dt.bfloat16
f32 = mybir.dt.float32
AF = mybir.ActivationFunctionType
BF = ml_dtypes.bfloat16

B, L, ENC, UNITS, EMB, VOCAB = 256, 64, 2048, 1024, 512, 50257
NCORES = 8
BC = B // NCORES            # 32 batch rows per core
TOK = BC * L                # 2048 tokens per core
GIN = ENC + EMB             # 2560
VS = 6656                   # padded vocab shard (13 * 512)
VPAD = VS * NCORES          # 53248
CORES = list(range(NCORES))


# ----------------------------------------------------------------------------
# Launch 1: attention + GRU + fc1, data-parallel over batch.
# ----------------------------------------------------------------------------
def build_l1(debug: bool = False):
    nc = bacc.Bacc("TRN2", target_bir_lowering=False, debug=False,
                   enable_asserts=True, num_devices=NCORES)
    dt = nc.dram_tensor
    ET = dt("ET", [ENC, TOK], bf16, kind="ExternalInput").ap()      # E^T (enc, tok)
    E = dt("E", [TOK, ENC], bf16, kind="ExternalInput").ap()        # E (tok, enc)
    W1 = dt("W1", [ENC, UNITS], bf16, kind="ExternalInput").ap()
    W2 = dt("W2", [UNITS, UNITS], bf16, kind="ExternalInput").ap()
    HIDT = dt("HIDT", [UNITS, BC], bf16, kind="ExternalInput").ap()  # hidden^T
    B12 = dt("B12", [1, UNITS], bf16, kind="ExternalInput").ap()     # b1+b2
    IND = dt("IND", [BC, TOK], bf16, kind="ExternalInput").ap()      # IND[b,t]=(t//L==b)
    V = dt("V", [UNITS, 1], bf16, kind="ExternalInput").ap()
    XET = dt("XET", [EMB, BC], bf16, kind="ExternalInput").ap()      # emb[X]^T
    GRUK = dt("GRUK", [GIN, 3 * UNITS], bf16, kind="ExternalInput").ap()
    GBROW = dt("GBROW", [1, 3 * UNITS], bf16, kind="ExternalInput").ap()  # folded gru bias
    HH = dt("HH", [1, UNITS], bf16, kind="ExternalInput").ap()       # gru_b[1][2U:3U]
    FC1 = dt("FC1", [UNITS, UNITS], bf16, kind="ExternalInput").ap()
    FC1B = dt("FC1B", [128, UNITS // 128], f32, kind="ExternalInput").ap()

    AW = dt("AW", [BC, L], f32, kind="ExternalOutput").ap()
    FS = dt("FS", [BC, UNITS], f32, kind="ExternalOutput").ap()
    O1T = dt("O1T", [UNITS, BC], bf16, kind="ExternalOutput").ap()
    if debug:
        DSC = dt("DSC", [BC, L], f32, kind="ExternalOutput").ap()       # raw score
        DHPP = dt("DHPP", [BC, UNITS], bf16, kind="ExternalOutput").ap()
        DXT = dt("DXT", [GIN, BC], bf16, kind="ExternalOutput").ap()    # x^T
        DXM = dt("DXM", [BC, 3 * UNITS], f32, kind="ExternalOutput").ap()

    KT = ENC // 128      # 16 k-tiles over enc
    MT = UNITS // 128    # 8 m-tiles over units
    NTK = TOK // 512     # 4 n-tiles over tokens

    with tile.TileContext(nc) as tc:
        with tc.tile_pool(name="persist", bufs=1) as pp:
            # ---- persistent small tiles
            ident = pp.tile([128, 128], bf16, tag="ident")
            make_identity(nc, ident)
            ones32 = pp.tile([1, BC], bf16, tag="ones32")
            nc.vector.memset(ones32[:], 1.0)
            b12 = pp.tile([1, UNITS], bf16, tag="b12")
            nc.sync.dma_start(b12[:], B12[:])
            indt = pp.tile([BC, TOK], bf16, tag="ind")
            nc.sync.dma_start(indt[:], IND[:])
            hid = []
            for k in range(MT):
                t = pp.tile([128, BC], bf16, tag=f"hid{k}")
                nc.sync.dma_start(t[:], HIDT[128 * k:128 * (k + 1), :])
                hid.append(t)
            vt = []
            for m in range(MT):
                t = pp.tile([128, 1], bf16, tag=f"v{m}")
                nc.sync.dma_start(t[:], V[128 * m:128 * (m + 1), :])
                vt.append(t)
            hpp = pp.tile([BC, UNITS], bf16, tag="hpp")
            gbrow = pp.tile([1, 3 * UNITS], bf16, tag="gbrow")
            nc.sync.dma_start(gbrow[:], GBROW[:])
            hhrow = pp.tile([1, UNITS], bf16, tag="hh")
            nc.sync.dma_start(hhrow[:], HH[:])
            fc1b = pp.tile([128, UNITS // 128], f32, tag="fc1b")
            nc.sync.dma_start(fc1b[:], FC1B[:])
            # x^T tiles: 16 context + 4 embedding
            xt = []
            for k in range(GIN // 128):
                xt.append(pp.tile([128, BC], bf16, tag=f"xt{k}", name=f"xt{k}"))
            for j in range(EMB // 128):
                nc.sync.dma_start(xt[KT + j][:], XET[128 * j:128 * (j + 1), :])

            # ---- H'' = hidden @ W2 + (b1+b2)   -> hpp [BC, UNITS] bf16
            with tc.tile_pool(name="w2p", bufs=1) as w2p, \
                 tc.tile_pool(name="psh", bufs=2, space="PSUM") as psh:
                w2t = []
                for k in range(MT):
                    t = w2p.tile([128, UNITS], bf16, tag=f"w2_{k}")
                    nc.sync.dma_start(t[:], W2[128 * k:128 * (k + 1), :])
                    w2t.append(t)
                for h in range(2):
                    ps = psh.tile([BC, 512], f32, tag="psh")
                    sl = slice(512 * h, 512 * (h + 1))
                    for k in range(MT):
                        nc.tensor.matmul(ps[:], hid[k][:], w2t[k][:, sl],
                                         start=(k == 0), stop=False)
                    nc.tensor.matmul(ps[:], ones32[:], b12[:, sl],
                                     start=False, stop=True)
                    nc.scalar.copy(hpp[:, sl], ps[:])
            if debug:
                nc.sync.dma_start(DHPP[:], hpp[:])

            # ---- phase A: S^T = tanh(W1^T ET + H''-ext), score = v^T S^T
            sv = pp.tile([1, TOK], f32, tag="sv")
            with tc.tile_pool(name="w1p", bufs=1) as w1p, \
                 tc.tile_pool(name="etp", bufs=2) as etp, \
                 tc.tile_pool(name="stp", bufs=2) as stp, \
                 tc.tile_pool(name="psA", bufs=3, space="PSUM") as psA, \
                 tc.tile_pool(name="psV", bufs=2, space="PSUM") as psV:
                w1t = []
                for k in range(KT):
                    t = w1p.tile([128, UNITS], bf16, tag=f"w1_{k}")
                    nc.sync.dma_start(t[:], W1[128 * k:128 * (k + 1), :])
                    w1t.append(t)
                for n in range(NTK):
                    nsl = slice(512 * n, 512 * (n + 1))
                    ett = []
                    for k in range(KT):
                        t = etp.tile([128, 512], bf16, tag=f"et{k}")
                        nc.sync.dma_start(t[:], ET[128 * k:128 * (k + 1), nsl])
                        ett.append(t)
                    sts = []
                    for m in range(MT):
                        msl = slice(128 * m, 128 * (m + 1))
                        ps = psA.tile([128, 512], f32, tag="psA")
                        for k in range(KT):
                            nc.tensor.matmul(ps[:], w1t[k][:, msl], ett[k][:],
                                             start=(k == 0), stop=False)
                        nc.tensor.matmul(ps[:], hpp[:, msl], indt[:, nsl],
                                         start=False, stop=True)
                        st = stp.tile([128, 512], bf16, tag=f"st{m}")
                        nc.scalar.activation(st[:], ps[:], AF.Tanh)
                        sts.append(st)
                    pv = psV.tile([1, 512], f32, tag="psV")
                    for m in range(MT):
                        nc.tensor.matmul(pv[:], vt[m][:], sts[m][:],
                                         start=(m == 0), stop=(m == MT - 1))
                    nc.scalar.copy(sv[:, nsl], pv[:])

            # ---- softmax over L within each batch row
            # route [1,2048] -> [32,64] reshape through DRAM (unambiguous)
            scratch = nc.dram_tensor("SCRATCH", [BC, L], f32).ap()
            nc.sync.dma_start(scratch[:], sv[:])
            sc = pp.tile([BC, L], f32, tag="sc")
            nc.sync.dma_start(sc[:], scratch[:])
            if debug:
                nc.sync.dma_start(DSC[:], sc[:])
            negmax = pp.tile([BC, 1], f32, tag="negmax")
            nc.vector.tensor_reduce(negmax[:], sc[:], axis=mybir.AxisListType.X,
                                    op=mybir.AluOpType.max, negate=True)
            ex = pp.tile([BC, L], f32, tag="ex")
            sumex = pp.tile([BC, 1], f32, tag="sumex")
            nc.scalar.activation(ex[:], sc[:], AF.Exp, bias=negmax[:],
                                 scale=1.0, accum_out=sumex[:])
            rec = pp.tile([BC, 1], f32, tag="rec")
            nc.vector.reciprocal(rec[:], sumex[:])
            aw = pp.tile([BC, L], f32, tag="aw")
            nc.vector.tensor_scalar_mul(aw[:], ex[:], rec[:])
            nc.sync.dma_start(AW[:], aw[:])

            # ---- W_blk [tok-tiles, 32] from attention weights
            wtd = pp.tile([128, BC], f32, tag="wtd")   # wT duplicated over halves
            nc.vector.transpose(wtd[0:32, :], aw[:, 0:32])
            nc.vector.transpose(wtd[32:64, :], aw[:, 32:64])
            # duplicate rows 0:64 into 64:128 via DMA (cross-partition move)
            nc.sync.dma_start(wtd[64:128, :], wtd[0:64, :])
            wblk = pp.tile([128, 16 * BC], bf16, tag="wblk")
            nc.vector.memset(wblk[:], 0.0)
            nc.vector.tensor_copy(wblk[0:64, 0:512:34], wtd[0:64, 0:32:2])
            nc.vector.tensor_copy(wblk[64:128, 1:512:34], wtd[64:128, 1:32:2])

            # ---- context^T: for each enc tile accumulate over tok tiles
            with tc.tile_pool(name="ep", bufs=1) as ep, \
                 tc.tile_pool(name="psC", bufs=3, space="PSUM") as psC:
                et_full = []
                for k in range(TOK // 128):
                    t = ep.tile([128, ENC], bf16, tag=f"e{k}")
                    nc.sync.dma_start(t[:], E[128 * k:128 * (k + 1), :])
                    et_full.append(t)
                for m in range(ENC // 128):
                    msl = slice(128 * m, 128 * (m + 1))
                    ps = psC.tile([128, BC], f32, tag="psC")
                    for k in range(TOK // 128):
                        nc.tensor.matmul(ps[:], et_full[k][:, msl],
                                         wblk[:, BC * k:BC * (k + 1)],
                                         start=(k == 0), stop=(k == TOK // 128 - 1))
                    nc.scalar.copy(xt[m][:], ps[:])
            if debug:
                for k in range(GIN // 128):
                    nc.sync.dma_start(DXT[128 * k:128 * (k + 1), :], xt[k][:])

            # ---- GRU: xm = x @ gru_k + bias-ext  (out [BC, 3U])
            zsb = pp.tile([BC, UNITS], f32, tag="zsb")
            rsb = pp.tile([BC, UNITS], f32, tag="rsb")
            xhsb = pp.tile([BC, UNITS], f32, tag="xhsb")
            with tc.tile_pool(name="gkp", bufs=4) as gkp, \
                 tc.tile_pool(name="psG", bufs=2, space="PSUM") as psG:
                for n in range(3 * UNITS // 512):
                    nsl = slice(512 * n, 512 * (n + 1))
                    ps = psG.tile([BC, 512], f32, tag="psG")
                    for k in range(GIN // 128):
                        gk = gkp.tile([128, 512], bf16, tag="gk")
                        nc.sync.dma_start(gk[:], GRUK[128 * k:128 * (k + 1), nsl])
                        nc.tensor.matmul(ps[:], xt[k][:], gk[:],
                                         start=(k == 0), stop=False)
                    nc.tensor.matmul(ps[:], ones32[:], gbrow[:, nsl],
                                     start=False, stop=True)
                    osl = slice(512 * (n % 2), 512 * (n % 2 + 1))
                    if n < 2:
                        nc.scalar.activation(zsb[:, osl], ps[:], AF.Sigmoid)
                    elif n < 4:
                        nc.scalar.activation(rsb[:, osl], ps[:], AF.Sigmoid)
                    else:
                        nc.scalar.copy(xhsb[:, osl], ps[:])
            if debug:
                nc.sync.dma_start(DXM[:, 0:UNITS], zsb[:])
                nc.sync.dma_start(DXM[:, UNITS:2 * UNITS], rsb[:])
                nc.sync.dma_start(DXM[:, 2 * UNITS:3 * UNITS], xhsb[:])

            cand = pp.tile([BC, UNITS], f32, tag="cand")
            with tc.tile_pool(name="psHH", bufs=2, space="PSUM") as psHH:
                for h in range(2):
                    sl = slice(512 * h, 512 * (h + 1))
                    ph = psHH.tile([BC, 512], f32, tag="psHH")
                    nc.tensor.matmul(ph[:], ones32[:], hhrow[:, sl],
                                     start=True, stop=True)
                    # rh = r * hh ; cand_pre = xh + rh
                    rh = pp.tile([BC, 512], f32, tag=f"rh{h}")
                    nc.vector.tensor_tensor(rh[:], rsb[:, sl], ph[:],
                                            op=mybir.AluOpType.mult)
                    nc.vector.tensor_tensor(rh[:], xhsb[:, sl], rh[:],
                                            op=mybir.AluOpType.add)
                    nc.scalar.activation(cand[:, sl], rh[:], AF.Tanh)
            onem = pp.tile([BC, UNITS], f32, tag="onem")
            nc.scalar.activation(onem[:], zsb[:], AF.Copy, bias=1.0, scale=-1.0)
            fs = pp.tile([BC, UNITS], f32, tag="fs")
            nc.vector.tensor_tensor(fs[:], onem[:], cand[:],
                                    op=mybir.AluOpType.mult)
            nc.sync.dma_start(FS[:], fs[:])
            fsb = pp.tile([BC, UNITS], bf16, tag="fsb")
            nc.vector.tensor_copy(fsb[:], fs[:])

            # ---- fs^T via PE transpose, then o1^T = fc1^T fs^T + b
            with tc.tile_pool(name="f1p", bufs=1) as f1p, \
                 tc.tile_pool(name="pst", bufs=2, space="PSUM") as pst, \
                 tc.tile_pool(name="psO", bufs=2, space="PSUM") as psO:
                fst = []
                for j in range(MT):
                    ps = pst.tile([128, BC], bf16, tag="pst")
                    nc.tensor.transpose(ps[:], fsb[:, 128 * j:128 * (j + 1)],
                                        ident[0:BC, 0:BC])
                    t = pp.tile([128, BC], bf16, tag=f"fst{j}")
                    nc.scalar.copy(t[:], ps[:])
                    fst.append(t)
                f1t = []
                for k in range(MT):
                    t = f1p.tile([128, UNITS], bf16, tag=f"f1_{k}")
                    nc.sync.dma_start(t[:], FC1[128 * k:128 * (k + 1), :])
                    f1t.append(t)
                for m in range(MT):
                    msl = slice(128 * m, 128 * (m + 1))
                    ps = psO.tile([128, BC], f32, tag="psO")
                    for k in range(MT):
                        nc.tensor.matmul(ps[:], f1t[k][:, msl], fst[k][:],
                                         start=(k == 0), stop=(k == MT - 1))
                    ot = pp.tile([128, BC], bf16, tag=f"o1t{m}")
                    nc.scalar.activation(ot[:], ps[:], AF.Identity,
                                         bias=fc1b[:, m:m + 1])
                    nc.sync.dma_start(O1T[128 * m:128 * (m + 1), :], ot[:])
    nc.compile()
    return nc


# ----------------------------------------------------------------------------
# Launch 2: logits = o1 @ fc2_w + fc2_b, vocab-parallel.
# ----------------------------------------------------------------------------
def build_l2():
    nc = bacc.Bacc("TRN2", target_bir_lowering=False, debug=False,
                   enable_asserts=True, num_devices=NCORES)
    dt = nc.dram_tensor
    O1TF = dt("O1TF", [UNITS, B], bf16, kind="ExternalInput").ap()
    FC2 = dt("FC2", [UNITS, VS], bf16, kind="ExternalInput").ap()
    FC2B = dt("FC2B", [1, VS], bf16, kind="ExternalInput").ap()
    LG = dt("LG", [B, VS], f32, kind="ExternalOutput").ap()

    KT = UNITS // 128   # 8
    NV = VS // 512      # 13

    with tile.TileContext(nc) as tc:
        with tc.tile_pool(name="pp", bufs=1) as pp, \
             tc.tile_pool(name="fc2p", bufs=1) as fc2p, \
             tc.tile_pool(name="outp", bufs=3) as outp, \
             tc.tile_pool(name="ps", bufs=1, space="PSUM") as psp:
            ones = pp.tile([1, 128], bf16, tag="ones")
            nc.vector.memset(ones[:], 1.0)
            b2row = pp.tile([1, VS], bf16, tag="b2row")
            nc.sync.dma_start(b2row[:], FC2B[:])
            o1k = []
            for k in range(KT):
                t = pp.tile([128, B], bf16, tag=f"o1_{k}")
                nc.sync.dma_start(t[:], O1TF[128 * k:128 * (k + 1), :])
                o1k.append(t)
            f2t = []
            for k in range(KT):
                t = fc2p.tile([128, VS], bf16, tag=f"f2_{k}")
                nc.sync.dma_start(t[:], FC2[128 * k:128 * (k + 1), :])
                f2t.append(t)
            for h in range(2):
                hsl = slice(128 * h, 128 * (h + 1))
                for chunk in ((0, 7), (7, 13)):
                    pss = {}
                    for vt in range(*chunk):
                        pss[vt] = psp.tile([128, 512], f32, tag=f"ps{vt - chunk[0]}", name=f"ps{vt}")
                    for k in range(KT):
                        for vt in range(*chunk):
                            vsl = slice(512 * vt, 512 * (vt + 1))
                            nc.tensor.matmul(pss[vt][:], o1k[k][:, hsl],
                                             f2t[k][:, vsl],
                                             start=(k == 0), stop=False)
                    for vt in range(*chunk):
                        vsl = slice(512 * vt, 512 * (vt + 1))
                        nc.tensor.matmul(pss[vt][:], ones[:], b2row[:, vsl],
                                         start=False, stop=True)
                        so = outp.tile([128, 512], f32, tag="so")
                        nc.scalar.copy(so[:], pss[vt][:])
                        nc.sync.dma_start(LG[hsl, vsl], so[:])
    nc.compile()
    return nc


# ----------------------------------------------------------------------------
# Host-side prep, launches, gather.
# ----------------------------------------------------------------------------
_NC_CACHE = {}


def _get_nc(name):
    if name not in _NC_CACHE:
        _NC_CACHE[name] = {"l1": build_l1, "l2": build_l2}[name]()
    return _NC_CACHE[name]


def _bf(a):
    return np.ascontiguousarray(np.asarray(a, np.float32)).astype(BF)


def prep_l1_maps(X, encoderOut, hidden, emb, W1, b1, W2, b2, v,
                 gru_b, fc1_w, fc1_b):
    X = np.asarray(X).astype(np.int64).reshape(B)
    enc = np.asarray(encoderOut, np.float32)
    hid = np.asarray(hidden, np.float32)
    emb = np.asarray(emb, np.float32)
    W1b = _bf(W1)
    W2b = _bf(W2)
    vb = _bf(np.asarray(v, np.float32).reshape(UNITS, 1))
    b12 = _bf((np.asarray(b1, np.float32) + np.asarray(b2, np.float32))
              .reshape(1, UNITS))
    ind = _bf(np.kron(np.eye(BC, dtype=np.float32), np.ones((1, L), np.float32)))
    gb = np.asarray(gru_b, np.float32)
    gbrow = np.concatenate([gb[0, :2 * UNITS] + gb[1, :2 * UNITS],
                            gb[0, 2 * UNITS:]]).reshape(1, 3 * UNITS)
    gbrow = _bf(gbrow)
    hh = _bf(gb[1, 2 * UNITS:].reshape(1, UNITS))
    fc1b = np.ascontiguousarray(
        np.asarray(fc1_b, np.float32).reshape(UNITS // 128, 128).T)
    fc1wb = _bf(fc1_w)

    maps = []
    for c in range(NCORES):
        rows = slice(BC * c, BC * (c + 1))
        Ec = np.ascontiguousarray(enc[rows].reshape(TOK, ENC))
        maps.append(dict(
            ET=_bf(Ec.T), E=_bf(Ec), W1=W1b, W2=W2b,
            HIDT=_bf(hid[rows].T), B12=b12, IND=ind, V=vb,
            XET=_bf(emb[X[rows]].T),
            GBROW=gbrow, HH=hh, FC1=fc1wb, FC1B=fc1b,
        ))
    return maps


def kernel(X, encoderOut, hidden, emb, W1, b1, W2, b2, v, bv,
           gru_k, gru_rk, gru_b, fc1_w, fc1_b, fc2_w, fc2_b):
    # --- launch 1 ---
    maps1 = prep_l1_maps(X, encoderOut, hidden, emb, W1, b1, W2, b2, v,
                         gru_b, fc1_w, fc1_b)
    gkb = _bf(gru_k)
    for m in maps1:
        m["GRUK"] = gkb
    nc1 = _get_nc("l1")
    res1 = run_bass_kernel_spmd(nc1, maps1, core_ids=CORES).results

    # --- gather o1^T, prep launch 2 ---
    o1t_full = np.concatenate([np.asarray(r["O1T"]) for r in res1], axis=1)
    o1t_full = np.ascontiguousarray(o1t_full).astype(BF, copy=False)
    fc2 = np.zeros((UNITS, VPAD), np.float32)
    fc2[:, :VOCAB] = np.asarray(fc2_w, np.float32)
    fc2 = fc2.astype(BF)
    fc2b = np.zeros((1, VPAD), np.float32)
    fc2b[0, :VOCAB] = np.asarray(fc2_b, np.float32)
    fc2b = fc2b.astype(BF)
    maps2 = []
    for c in range(NCORES):
        csl = slice(VS * c, VS * (c + 1))
        maps2.append(dict(O1TF=o1t_full,
                          FC2=np.ascontiguousarray(fc2[:, csl]),
                          FC2B=np.ascontiguousarray(fc2b[:, csl])))
    nc2 = _get_nc("l2")
    res2 = run_bass_kernel_spmd(nc2, maps2, core_ids=CORES).results

    # --- gather outputs ---
    logits = np.concatenate([np.asarray(r["LG"], np.float32) for r in res2],
                            axis=1)[:, :VOCAB]
    finalState = np.concatenate([np.asarray(r["FS"], np.float32) for r in res1],
                                axis=0)
    attenWts = np.concatenate([np.asarray(r["AW"], np.float32) for r in res1],
                              axis=0).reshape(B, L, 1)
    return logits, finalState, attenWts


# revision 7
# speedup vs baseline: 1.1618x; 1.1618x over previous
"""Trainium2 Bass kernel for nn_Decoder_57758720196948.

Bahdanau-attention decoder step:
  score   = tanh(encoderOut @ W1 + b1 + (hidden @ W2 + b2)[:,None,:])
  attenW  = softmax(score @ v + bv, axis=L)          (bv drops out of softmax)
  context = sum(attenW * encoderOut, axis=L)
  x       = [context ; emb[X]]
  GRU with h_prev = 0  =>  hm = gru_b[1], gru_rk unused,
      z = sigmoid(xz+hz), r = sigmoid(xr+hr), cand = tanh(xh + r*hh),
      h' = (1-z)*cand
  o1      = h' @ fc1_w + fc1_b
  logits  = o1 @ fc2_w + fc2_b

Sharding: launch 1 is data-parallel over batch (32 rows/core); launch 2 is
tensor-parallel over the vocab axis of fc2 (6656 padded cols/core), with the
gather of logits done host-side. Matmuls run in bf16 with fp32 PSUM
accumulation; softmax/GRU elementwise math in fp32.
"""
import numpy as np
import ml_dtypes

import concourse.bass as bass
import concourse.tile as tile
from concourse import bacc, mybir
from concourse.bass_utils import run_bass_kernel_spmd
from concourse.masks import make_identity

bf16 = mybir.dt.bfloat16
f32 = mybir.dt.float32
AF = mybir.ActivationFunctionType
BF = ml_dtypes.bfloat16

B, L, ENC, UNITS, EMB, VOCAB = 256, 64, 2048, 1024, 512, 50257
NCORES = 8
BC = B // NCORES            # 32 batch rows per core
TOK = BC * L                # 2048 tokens per core
GIN = ENC + EMB             # 2560
VS = 6656                   # padded vocab shard (13 * 512)
VPAD = VS * NCORES          # 53248
CORES = list(range(NCORES))


# ----------------------------------------------------------------------------
# Launch 1: attention + GRU + fc1, data-parallel over batch.
# ----------------------------------------------------------------------------
def build_l1(debug: bool = False):
    nc = bacc.Bacc("TRN2", target_bir_lowering=False, debug=False,
                   enable_asserts=True, num_devices=NCORES)
    dt = nc.dram_tensor
    ET = dt("ET", [ENC, TOK], bf16, kind="ExternalInput").ap()      # E^T (enc, tok)
    E = dt("E", [TOK, ENC], bf16, kind="ExternalInput").ap()        # E (tok, enc)
    W1 = dt("W1", [ENC, UNITS], bf16, kind="ExternalInput").ap()
    W2 = dt("W2", [UNITS, UNITS], bf16, kind="ExternalInput").ap()
    HIDT = dt("HIDT", [UNITS, BC], bf16, kind="ExternalInput").ap()  # hidden^T
    B12 = dt("B12", [1, UNITS], bf16, kind="ExternalInput").ap()     # b1+b2
    IND = dt("IND", [BC, TOK], bf16, kind="ExternalInput").ap()      # IND[b,t]=(t//L==b)
    V = dt("V", [UNITS, 1], bf16, kind="ExternalInput").ap()
    XET = dt("XET", [EMB, BC], bf16, kind="ExternalInput").ap()      # emb[X]^T
    GRUK = dt("GRUK", [GIN, 3 * UNITS], bf16, kind="ExternalInput").ap()
    GBROW = dt("GBROW", [1, 3 * UNITS], bf16, kind="ExternalInput").ap()  # folded gru bias
    HH = dt("HH", [1, UNITS], bf16, kind="ExternalInput").ap()       # gru_b[1][2U:3U]
    FC1 = dt("FC1", [UNITS, UNITS], bf16, kind="ExternalInput").ap()
    FC1B = dt("FC1B", [128, UNITS // 128], f32, kind="ExternalInput").ap()

    AW = dt("AW", [BC, L], f32, kind="ExternalOutput").ap()
    FS = dt("FS", [BC, UNITS], f32, kind="ExternalOutput").ap()
    O1T = dt("O1T", [UNITS, BC], bf16, kind="ExternalOutput").ap()
    if debug:
        DSC = dt("DSC", [BC, L], f32, kind="ExternalOutput").ap()       # raw score
        DHPP = dt("DHPP", [BC, UNITS], bf16, kind="ExternalOutput").ap()
        DXT = dt("DXT", [GIN, BC], bf16, kind="ExternalOutput").ap()    # x^T
        DXM = dt("DXM", [BC, 3 * UNITS], f32, kind="ExternalOutput").ap()

    KT = ENC // 128      # 16 k-tiles over enc
    MT = UNITS // 128    # 8 m-tiles over units
    NTK = TOK // 512     # 4 n-tiles over tokens

    with tile.TileContext(nc) as tc:
        with tc.tile_pool(name="persist", bufs=1) as pp:
            # ---- persistent small tiles
            ones32 = pp.tile([1, BC], bf16, tag="ones32")
            nc.vector.memset(ones32[:], 1.0)
            b12 = pp.tile([1, UNITS], bf16, tag="b12")
            nc.sync.dma_start(b12[:], B12[:])
            hid = []
            for k in range(MT):
                t = pp.tile([128, BC], bf16, tag=f"hid{k}")
                nc.sync.dma_start(t[:], HIDT[128 * k:128 * (k + 1), :])
                hid.append(t)
            ident = pp.tile([128, 128], bf16, tag="ident")
            make_identity(nc, ident)
            indt = pp.tile([BC, TOK], bf16, tag="ind")
            nc.sync.dma_start(indt[:], IND[:])
            vt = []
            for m in range(MT):
                t = pp.tile([128, 1], bf16, tag=f"v{m}")
                nc.sync.dma_start(t[:], V[128 * m:128 * (m + 1), :])
                vt.append(t)
            hpp = pp.tile([BC, UNITS], bf16, tag="hpp")
            gbrow = pp.tile([1, 3 * UNITS], bf16, tag="gbrow")
            nc.sync.dma_start(gbrow[:], GBROW[:])
            hhrow = pp.tile([1, UNITS], bf16, tag="hh")
            nc.sync.dma_start(hhrow[:], HH[:])
            fc1b = pp.tile([128, UNITS // 128], f32, tag="fc1b")
            nc.sync.dma_start(fc1b[:], FC1B[:])
            # x^T tiles: 16 context + 4 embedding
            xt = []
            for k in range(GIN // 128):
                xt.append(pp.tile([128, BC], bf16, tag=f"xt{k}", name=f"xt{k}"))
            for j in range(EMB // 128):
                nc.sync.dma_start(xt[KT + j][:], XET[128 * j:128 * (j + 1), :])

            # ---- H'' = hidden @ W2 + (b1+b2)   -> hpp [BC, UNITS] bf16
            with tc.tile_pool(name="w2p", bufs=1) as w2p, \
                 tc.tile_pool(name="psh", bufs=2, space="PSUM") as psh:
                w2t = []
                for k in range(MT):
                    t = w2p.tile([128, UNITS], bf16, tag=f"w2_{k}")
                    nc.sync.dma_start(t[:], W2[128 * k:128 * (k + 1), :])
                    w2t.append(t)
                for h in range(2):
                    ps = psh.tile([BC, 512], f32, tag="psh")
                    sl = slice(512 * h, 512 * (h + 1))
                    for k in range(MT):
                        nc.tensor.matmul(ps[:], hid[k][:], w2t[k][:, sl],
                                         start=(k == 0), stop=False)
                    nc.tensor.matmul(ps[:], ones32[:], b12[:, sl],
                                     start=False, stop=True)
                    nc.scalar.copy(hpp[:, sl], ps[:])
            if debug:
                nc.sync.dma_start(DHPP[:], hpp[:])

            # ---- phase A: S^T = tanh(W1^T ET + H''-ext), score = v^T S^T
            sv = pp.tile([1, TOK], f32, tag="sv")
            ep_cm = tc.tile_pool(name="ep", bufs=1)
            ep = ep_cm.__enter__()
            with tc.tile_pool(name="w1p", bufs=1) as w1p, \
                 tc.tile_pool(name="etp", bufs=2) as etp, \
                 tc.tile_pool(name="stp", bufs=2) as stp, \
                 tc.tile_pool(name="psA", bufs=3, space="PSUM") as psA, \
                 tc.tile_pool(name="psV", bufs=2, space="PSUM") as psV:
                w1t = []
                for k in range(KT):
                    t = w1p.tile([128, UNITS], bf16, tag=f"w1_{k}")
                    nc.sync.dma_start(t[:], W1[128 * k:128 * (k + 1), :])
                    w1t.append(t)
                # hoist E loads so they land during phase A compute
                et_full = []
                for k in range(TOK // 128):
                    t = ep.tile([128, ENC], bf16, tag=f"e{k}", name=f"e{k}")
                    nc.sync.dma_start(t[:], E[128 * k:128 * (k + 1), :])
                    et_full.append(t)
                for n in range(NTK):
                    nsl = slice(512 * n, 512 * (n + 1))
                    ett = []
                    for k in range(KT):
                        t = etp.tile([128, 512], bf16, tag=f"et{k}")
                        nc.sync.dma_start(t[:], ET[128 * k:128 * (k + 1), nsl])
                        ett.append(t)
                    sts = []
                    for m in range(MT):
                        msl = slice(128 * m, 128 * (m + 1))
                        ps = psA.tile([128, 512], f32, tag="psA")
                        for k in range(KT):
                            nc.tensor.matmul(ps[:], w1t[k][:, msl], ett[k][:],
                                             start=(k == 0), stop=False)
                        nc.tensor.matmul(ps[:], hpp[:, msl], indt[:, nsl],
                                         start=False, stop=True)
                        st = stp.tile([128, 512], bf16, tag=f"st{m}")
                        nc.scalar.activation(st[:], ps[:], AF.Tanh)
                        sts.append(st)
                    pv = psV.tile([1, 512], f32, tag="psV")
                    for m in range(MT):
                        nc.tensor.matmul(pv[:], vt[m][:], sts[m][:],
                                         start=(m == 0), stop=(m == MT - 1))
                    nc.scalar.copy(sv[:, nsl], pv[:])

            # ---- softmax over L within each batch row
            # route [1,2048] -> [32,64] reshape through DRAM (unambiguous)
            scratch = nc.dram_tensor("SCRATCH", [BC, L], f32).ap()
            nc.sync.dma_start(scratch[:], sv[:])
            sc = pp.tile([BC, L], f32, tag="sc")
            nc.sync.dma_start(sc[:], scratch[:])
            if debug:
                nc.sync.dma_start(DSC[:], sc[:])
            negmax = pp.tile([BC, 1], f32, tag="negmax")
            nc.vector.tensor_reduce(negmax[:], sc[:], axis=mybir.AxisListType.X,
                                    op=mybir.AluOpType.max, negate=True)
            ex = pp.tile([BC, L], f32, tag="ex")
            sumex = pp.tile([BC, 1], f32, tag="sumex")
            nc.scalar.activation(ex[:], sc[:], AF.Exp, bias=negmax[:],
                                 scale=1.0, accum_out=sumex[:])
            rec = pp.tile([BC, 1], f32, tag="rec")
            nc.vector.reciprocal(rec[:], sumex[:])
            aw = pp.tile([BC, L], f32, tag="aw")
            nc.vector.tensor_scalar_mul(aw[:], ex[:], rec[:])
            nc.sync.dma_start(AW[:], aw[:])

            # ---- W_blk [tok-tiles, 32] from attention weights
            wtd = pp.tile([128, BC], f32, tag="wtd")   # wT duplicated over halves
            nc.vector.transpose(wtd[0:32, :], aw[:, 0:32])
            nc.vector.transpose(wtd[32:64, :], aw[:, 32:64])
            # duplicate rows 0:64 into 64:128 via DMA (cross-partition move)
            nc.sync.dma_start(wtd[64:128, :], wtd[0:64, :])
            wblk = pp.tile([128, 16 * BC], bf16, tag="wblk")
            nc.vector.memset(wblk[:], 0.0)
            nc.vector.tensor_copy(wblk[0:64, 0:512:34], wtd[0:64, 0:32:2])
            nc.vector.tensor_copy(wblk[64:128, 1:512:34], wtd[64:128, 1:32:2])

            # ---- context^T: for each enc tile accumulate over tok tiles
            with tc.tile_pool(name="psC", bufs=3, space="PSUM") as psC:
                for m in range(ENC // 128):
                    msl = slice(128 * m, 128 * (m + 1))
                    ps = psC.tile([128, BC], f32, tag="psC")
                    for k in range(TOK // 128):
                        nc.tensor.matmul(ps[:], et_full[k][:, msl],
                                         wblk[:, BC * k:BC * (k + 1)],
                                         start=(k == 0), stop=(k == TOK // 128 - 1))
                    nc.scalar.copy(xt[m][:], ps[:])
            ep_cm.__exit__(None, None, None)
            if debug:
                for k in range(GIN // 128):
                    nc.sync.dma_start(DXT[128 * k:128 * (k + 1), :], xt[k][:])

            # ---- GRU: xm = x @ gru_k + bias-ext  (out [BC, 3U])
            zsb = pp.tile([BC, UNITS], f32, tag="zsb")
            rsb = pp.tile([BC, UNITS], f32, tag="rsb")
            xhsb = pp.tile([BC, UNITS], f32, tag="xhsb")
            NG = 3 * UNITS // 512      # 6 n-tiles
            with tc.tile_pool(name="gkp", bufs=4) as gkp, \
                 tc.tile_pool(name="psG", bufs=1, space="PSUM") as psG:
                pgs = [psG.tile([BC, 512], f32, tag=f"psG{n}", name=f"psG{n}")
                       for n in range(NG)]
                for k in range(GIN // 128):
                    gk = gkp.tile([128, 3 * UNITS], bf16, tag="gk")
                    nc.sync.dma_start(gk[:], GRUK[128 * k:128 * (k + 1), :])
                    for n in range(NG):
                        nc.tensor.matmul(pgs[n][:], xt[k][:],
                                         gk[:, 512 * n:512 * (n + 1)],
                                         start=(k == 0), stop=False)
                for n in range(NG):
                    nc.tensor.matmul(pgs[n][:], ones32[:],
                                     gbrow[:, 512 * n:512 * (n + 1)],
                                     start=False, stop=True)
                    osl = slice(512 * (n % 2), 512 * (n % 2 + 1))
                    if n < 2:
                        nc.scalar.activation(zsb[:, osl], pgs[n][:], AF.Sigmoid)
                    elif n < 4:
                        nc.scalar.activation(rsb[:, osl], pgs[n][:], AF.Sigmoid)
                    else:
                        nc.scalar.copy(xhsb[:, osl], pgs[n][:])
            if debug:
                nc.sync.dma_start(DXM[:, 0:UNITS], zsb[:])
                nc.sync.dma_start(DXM[:, UNITS:2 * UNITS], rsb[:])
                nc.sync.dma_start(DXM[:, 2 * UNITS:3 * UNITS], xhsb[:])

            cand = pp.tile([BC, UNITS], f32, tag="cand")
            with tc.tile_pool(name="psHH", bufs=2, space="PSUM") as psHH:
                for h in range(2):
                    sl = slice(512 * h, 512 * (h + 1))
                    ph = psHH.tile([BC, 512], f32, tag="psHH")
                    nc.tensor.matmul(ph[:], ones32[:], hhrow[:, sl],
                                     start=True, stop=True)
                    # rh = r * hh ; cand_pre = xh + rh
                    rh = pp.tile([BC, 512], f32, tag=f"rh{h}")
                    nc.vector.tensor_tensor(rh[:], rsb[:, sl], ph[:],
                                            op=mybir.AluOpType.mult)
                    nc.vector.tensor_tensor(rh[:], xhsb[:, sl], rh[:],
                                            op=mybir.AluOpType.add)
                    nc.scalar.activation(cand[:, sl], rh[:], AF.Tanh)
            onem = pp.tile([BC, UNITS], f32, tag="onem")
            nc.scalar.activation(onem[:], zsb[:], AF.Copy, bias=1.0, scale=-1.0)
            fs = pp.tile([BC, UNITS], f32, tag="fs")
            nc.vector.tensor_tensor(fs[:], onem[:], cand[:],
                                    op=mybir.AluOpType.mult)
            nc.sync.dma_start(FS[:], fs[:])
            fsb = pp.tile([BC, UNITS], bf16, tag="fsb")
            nc.vector.tensor_copy(fsb[:], fs[:])

            # ---- fs^T via PE transpose, then o1^T = fc1^T fs^T + b
            with tc.tile_pool(name="f1p", bufs=1) as f1p, \
                 tc.tile_pool(name="pst", bufs=2, space="PSUM") as pst, \
                 tc.tile_pool(name="psO", bufs=2, space="PSUM") as psO:
                fst = []
                for j in range(MT):
                    ps = pst.tile([128, BC], bf16, tag="pst")
                    nc.tensor.transpose(ps[:], fsb[:, 128 * j:128 * (j + 1)],
                                        ident[0:BC, 0:BC])
                    t = pp.tile([128, BC], bf16, tag=f"fst{j}")
                    nc.scalar.copy(t[:], ps[:])
                    fst.append(t)
                f1t = []
                for k in range(MT):
                    t = f1p.tile([128, UNITS], bf16, tag=f"f1_{k}")
                    nc.sync.dma_start(t[:], FC1[128 * k:128 * (k + 1), :])
                    f1t.append(t)
                for m in range(MT):
                    msl = slice(128 * m, 128 * (m + 1))
                    ps = psO.tile([128, BC], f32, tag="psO")
                    for k in range(MT):
                        nc.tensor.matmul(ps[:], f1t[k][:, msl], fst[k][:],
                                         start=(k == 0), stop=(k == MT - 1))
                    ot = pp.tile([128, BC], bf16, tag=f"o1t{m}")
                    nc.scalar.activation(ot[:], ps[:], AF.Identity,
                                         bias=fc1b[:, m:m + 1])
                    nc.sync.dma_start(O1T[128 * m:128 * (m + 1), :], ot[:])
    nc.compile()
    return nc


# ----------------------------------------------------------------------------
# Launch 2: logits = o1 @ fc2_w + fc2_b, vocab-parallel.
# ----------------------------------------------------------------------------
def build_l2():
    nc = bacc.Bacc("TRN2", target_bir_lowering=False, debug=False,
                   enable_asserts=True, num_devices=NCORES)
    dt = nc.dram_tensor
    O1TF = dt("O1TF", [UNITS, B], bf16, kind="ExternalInput").ap()
    FC2 = dt("FC2", [UNITS, VS], bf16, kind="ExternalInput").ap()
    FC2B = dt("FC2B", [1, VS], bf16, kind="ExternalInput").ap()
    LG = dt("LG", [B, VS], f32, kind="ExternalOutput").ap()

    KT = UNITS // 128   # 8
    NV = VS // 512      # 13

    with tile.TileContext(nc) as tc:
        with tc.tile_pool(name="pp", bufs=1) as pp, \
             tc.tile_pool(name="fc2p", bufs=1) as fc2p, \
             tc.tile_pool(name="outp", bufs=3) as outp, \
             tc.tile_pool(name="ps", bufs=1, space="PSUM") as psp:
            ones = pp.tile([1, 128], bf16, tag="ones")
            nc.vector.memset(ones[:], 1.0)
            b2row = pp.tile([1, VS], bf16, tag="b2row")
            nc.sync.dma_start(b2row[:], FC2B[:])
            o1k = []
            for k in range(KT):
                t = pp.tile([128, B], bf16, tag=f"o1_{k}")
                nc.sync.dma_start(t[:], O1TF[128 * k:128 * (k + 1), :])
                o1k.append(t)
            f2t = []
            for k in range(KT):
                t = fc2p.tile([128, VS], bf16, tag=f"f2_{k}")
                nc.sync.dma_start(t[:], FC2[128 * k:128 * (k + 1), :])
                f2t.append(t)
            for h in range(2):
                hsl = slice(128 * h, 128 * (h + 1))
                for chunk in ((0, 7), (7, 13)):
                    pss = {}
                    for vt in range(*chunk):
                        pss[vt] = psp.tile([128, 512], f32, tag=f"ps{vt - chunk[0]}", name=f"ps{vt}")
                    for k in range(KT):
                        for vt in range(*chunk):
                            vsl = slice(512 * vt, 512 * (vt + 1))
                            nc.tensor.matmul(pss[vt][:], o1k[k][:, hsl],
                                             f2t[k][:, vsl],
                                             start=(k == 0), stop=False)
                    for vt in range(*chunk):
                        vsl = slice(512 * vt, 512 * (vt + 1))
                        nc.tensor.matmul(pss[vt][:], ones[:], b2row[:, vsl],
                                         start=False, stop=True)
                        so = outp.tile([128, 512], f32, tag="so")
                        nc.scalar.copy(so[:], pss[vt][:])
                        nc.sync.dma_start(LG[hsl, vsl], so[:])
    nc.compile()
    return nc


# ----------------------------------------------------------------------------
# Host-side prep, launches, gather.
# ----------------------------------------------------------------------------
_NC_CACHE = {}


def _get_nc(name):
    if name not in _NC_CACHE:
        _NC_CACHE[name] = {"l1": build_l1, "l2": build_l2}[name]()
    return _NC_CACHE[name]


def _bf(a):
    return np.ascontiguousarray(np.asarray(a, np.float32)).astype(BF)


def prep_l1_maps(X, encoderOut, hidden, emb, W1, b1, W2, b2, v,
                 gru_b, fc1_w, fc1_b):
    X = np.asarray(X).astype(np.int64).reshape(B)
    enc = np.asarray(encoderOut, np.float32)
    hid = np.asarray(hidden, np.float32)
    emb = np.asarray(emb, np.float32)
    W1b = _bf(W1)
    W2b = _bf(W2)
    vb = _bf(np.asarray(v, np.float32).reshape(UNITS, 1))
    b12 = _bf((np.asarray(b1, np.float32) + np.asarray(b2, np.float32))
              .reshape(1, UNITS))
    ind = _bf(np.kron(np.eye(BC, dtype=np.float32), np.ones((1, L), np.float32)))
    gb = np.asarray(gru_b, np.float32)
    gbrow = np.concatenate([gb[0, :2 * UNITS] + gb[1, :2 * UNITS],
                            gb[0, 2 * UNITS:]]).reshape(1, 3 * UNITS)
    gbrow = _bf(gbrow)
    hh = _bf(gb[1, 2 * UNITS:].reshape(1, UNITS))
    fc1b = np.ascontiguousarray(
        np.asarray(fc1_b, np.float32).reshape(UNITS // 128, 128).T)
    fc1wb = _bf(fc1_w)

    maps = []
    for c in range(NCORES):
        rows = slice(BC * c, BC * (c + 1))
        Ec = np.ascontiguousarray(enc[rows].reshape(TOK, ENC))
        maps.append(dict(
            ET=_bf(Ec.T), E=_bf(Ec), W1=W1b, W2=W2b,
            HIDT=_bf(hid[rows].T), B12=b12, IND=ind, V=vb,
            XET=_bf(emb[X[rows]].T),
            GBROW=gbrow, HH=hh, FC1=fc1wb, FC1B=fc1b,
        ))
    return maps


def kernel(X, encoderOut, hidden, emb, W1, b1, W2, b2, v, bv,
           gru_k, gru_rk, gru_b, fc1_w, fc1_b, fc2_w, fc2_b):
    # --- launch 1 ---
    maps1 = prep_l1_maps(X, encoderOut, hidden, emb, W1, b1, W2, b2, v,
                         gru_b, fc1_w, fc1_b)
    gkb = _bf(gru_k)
    for m in maps1:
        m["GRUK"] = gkb
    nc1 = _get_nc("l1")
    res1 = run_bass_kernel_spmd(nc1, maps1, core_ids=CORES).results

    # --- gather o1^T, prep launch 2 ---
    o1t_full = np.concatenate([np.asarray(r["O1T"]) for r in res1], axis=1)
    o1t_full = np.ascontiguousarray(o1t_full).astype(BF, copy=False)
    fc2 = np.zeros((UNITS, VPAD), np.float32)
    fc2[:, :VOCAB] = np.asarray(fc2_w, np.float32)
    fc2 = fc2.astype(BF)
    fc2b = np.zeros((1, VPAD), np.float32)
    fc2b[0, :VOCAB] = np.asarray(fc2_b, np.float32)
    fc2b = fc2b.astype(BF)
    maps2 = []
    for c in range(NCORES):
        csl = slice(VS * c, VS * (c + 1))
        maps2.append(dict(O1TF=o1t_full,
                          FC2=np.ascontiguousarray(fc2[:, csl]),
                          FC2B=np.ascontiguousarray(fc2b[:, csl])))
    nc2 = _get_nc("l2")
    res2 = run_bass_kernel_spmd(nc2, maps2, core_ids=CORES).results

    # --- gather outputs ---
    logits = np.concatenate([np.asarray(r["LG"], np.float32) for r in res2],
                            axis=1)[:, :VOCAB]
    finalState = np.concatenate([np.asarray(r["FS"], np.float32) for r in res1],
                                axis=0)
    attenWts = np.concatenate([np.asarray(r["AW"], np.float32) for r in res1],
                              axis=0).reshape(B, L, 1)
    return logits, finalState, attenWts


# revision 8
# speedup vs baseline: 1.2803x; 1.1020x over previous
"""Trainium2 Bass kernel for nn_Decoder_57758720196948.

Bahdanau-attention decoder step:
  score   = tanh(encoderOut @ W1 + b1 + (hidden @ W2 + b2)[:,None,:])
  attenW  = softmax(score @ v + bv, axis=L)          (bv drops out of softmax)
  context = sum(attenW * encoderOut, axis=L)
  x       = [context ; emb[X]]
  GRU with h_prev = 0  =>  hm = gru_b[1], gru_rk unused,
      z = sigmoid(xz+hz), r = sigmoid(xr+hr), cand = tanh(xh + r*hh),
      h' = (1-z)*cand
  o1      = h' @ fc1_w + fc1_b
  logits  = o1 @ fc2_w + fc2_b

Sharding: launch 1 is data-parallel over batch (32 rows/core); launch 2 is
tensor-parallel over the vocab axis of fc2 (6656 padded cols/core), with the
gather of logits done host-side. Matmuls run in bf16 with fp32 PSUM
accumulation; softmax/GRU elementwise math in fp32.
"""
import numpy as np
import ml_dtypes

import concourse.bass as bass
import concourse.tile as tile
from concourse import bacc, mybir
from concourse.bass_utils import run_bass_kernel_spmd
from concourse.masks import make_identity

bf16 = mybir.dt.bfloat16
f32 = mybir.dt.float32
AF = mybir.ActivationFunctionType
BF = ml_dtypes.bfloat16

B, L, ENC, UNITS, EMB, VOCAB = 256, 64, 2048, 1024, 512, 50257
NCORES = 8
BC = B // NCORES            # 32 batch rows per core
TOK = BC * L                # 2048 tokens per core
GIN = ENC + EMB             # 2560
VS = 6656                   # padded vocab shard (13 * 512)
VPAD = VS * NCORES          # 53248
CORES = list(range(NCORES))


# ----------------------------------------------------------------------------
# Launch 1: attention + GRU + fc1, data-parallel over batch.
# ----------------------------------------------------------------------------
def build_l1(debug: bool = False):
    nc = bacc.Bacc("TRN2", target_bir_lowering=False, debug=False,
                   enable_asserts=True, num_devices=NCORES)
    dt = nc.dram_tensor
    ET = dt("ET", [ENC, TOK], bf16, kind="ExternalInput").ap()      # E^T (enc, tok)
    E = dt("E", [TOK, ENC], bf16, kind="ExternalInput").ap()        # E (tok, enc)
    W1 = dt("W1", [ENC, UNITS], bf16, kind="ExternalInput").ap()
    W2 = dt("W2", [UNITS, UNITS], bf16, kind="ExternalInput").ap()
    HIDT = dt("HIDT", [UNITS, BC], bf16, kind="ExternalInput").ap()  # hidden^T
    B12 = dt("B12", [1, UNITS], bf16, kind="ExternalInput").ap()     # b1+b2
    IND = dt("IND", [BC, TOK], bf16, kind="ExternalInput").ap()      # IND[b,t]=(t//L==b)
    V = dt("V", [UNITS, 1], bf16, kind="ExternalInput").ap()
    XET = dt("XET", [EMB, BC], bf16, kind="ExternalInput").ap()      # emb[X]^T
    GRUK = dt("GRUK", [GIN, 3 * UNITS], bf16, kind="ExternalInput").ap()
    GBROW = dt("GBROW", [1, 3 * UNITS], bf16, kind="ExternalInput").ap()  # folded gru bias
    HH = dt("HH", [1, UNITS], bf16, kind="ExternalInput").ap()       # gru_b[1][2U:3U]
    FC1 = dt("FC1", [UNITS, UNITS], bf16, kind="ExternalInput").ap()
    FC1B = dt("FC1B", [128, UNITS // 128], f32, kind="ExternalInput").ap()

    AW = dt("AW", [BC, L], f32, kind="ExternalOutput").ap()
    FS = dt("FS", [BC, UNITS], f32, kind="ExternalOutput").ap()
    O1T = dt("O1T", [UNITS, BC], bf16, kind="ExternalOutput").ap()
    if debug:
        DSC = dt("DSC", [BC, L], f32, kind="ExternalOutput").ap()       # raw score
        DHPP = dt("DHPP", [BC, UNITS], bf16, kind="ExternalOutput").ap()
        DXT = dt("DXT", [GIN, BC], bf16, kind="ExternalOutput").ap()    # x^T
        DXM = dt("DXM", [BC, 3 * UNITS], f32, kind="ExternalOutput").ap()

    KT = ENC // 128      # 16 k-tiles over enc
    MT = UNITS // 128    # 8 m-tiles over units
    NTK = TOK // 512     # 4 n-tiles over tokens

    with tile.TileContext(nc) as tc:
        with tc.tile_pool(name="persist", bufs=1) as pp:
            # ---- persistent small tiles
            ones32 = pp.tile([1, BC], bf16, tag="ones32")
            nc.vector.memset(ones32[:], 1.0)
            b12 = pp.tile([1, UNITS], bf16, tag="b12")
            nc.sync.dma_start(b12[:], B12[:])
            hid = []
            for k in range(MT):
                t = pp.tile([128, BC], bf16, tag=f"hid{k}")
                nc.sync.dma_start(t[:], HIDT[128 * k:128 * (k + 1), :])
                hid.append(t)
            ident = pp.tile([128, 128], bf16, tag="ident")
            make_identity(nc, ident)
            indt = pp.tile([BC, TOK], bf16, tag="ind")
            nc.sync.dma_start(indt[:], IND[:])
            vt = []
            for m in range(MT):
                t = pp.tile([128, 1], bf16, tag=f"v{m}")
                nc.sync.dma_start(t[:], V[128 * m:128 * (m + 1), :])
                vt.append(t)
            hpp = pp.tile([BC, UNITS], bf16, tag="hpp")
            gbrow = pp.tile([1, 3 * UNITS], bf16, tag="gbrow")
            nc.sync.dma_start(gbrow[:], GBROW[:])
            hhrow = pp.tile([1, UNITS], bf16, tag="hh")
            nc.sync.dma_start(hhrow[:], HH[:])
            fc1b = pp.tile([128, UNITS // 128], f32, tag="fc1b")
            nc.sync.dma_start(fc1b[:], FC1B[:])
            # x^T tiles: 16 context + 4 embedding
            xt = []
            for k in range(GIN // 128):
                xt.append(pp.tile([128, BC], bf16, tag=f"xt{k}", name=f"xt{k}"))
            for j in range(EMB // 128):
                nc.sync.dma_start(xt[KT + j][:], XET[128 * j:128 * (j + 1), :])

            # ---- H'' = hidden @ W2 + (b1+b2)   -> hpp [BC, UNITS] bf16
            with tc.tile_pool(name="w2p", bufs=1) as w2p, \
                 tc.tile_pool(name="psh", bufs=2, space="PSUM") as psh:
                w2t = []
                for k in range(MT):
                    t = w2p.tile([128, UNITS], bf16, tag=f"w2_{k}")
                    nc.sync.dma_start(t[:], W2[128 * k:128 * (k + 1), :])
                    w2t.append(t)
                for h in range(2):
                    ps = psh.tile([BC, 512], f32, tag="psh")
                    sl = slice(512 * h, 512 * (h + 1))
                    for k in range(MT):
                        nc.tensor.matmul(ps[:], hid[k][:], w2t[k][:, sl],
                                         start=(k == 0), stop=False)
                    nc.tensor.matmul(ps[:], ones32[:], b12[:, sl],
                                     start=False, stop=True)
                    nc.scalar.copy(hpp[:, sl], ps[:])
            if debug:
                nc.sync.dma_start(DHPP[:], hpp[:])

            # ---- phase A: S^T = tanh(W1^T ET + H''-ext), score = v^T S^T
            sv = pp.tile([1, TOK], f32, tag="sv")
            ep_cm = tc.tile_pool(name="ep", bufs=1)
            ep = ep_cm.__enter__()
            with tc.tile_pool(name="w1p", bufs=1) as w1p, \
                 tc.tile_pool(name="etp", bufs=2) as etp, \
                 tc.tile_pool(name="stp", bufs=2) as stp, \
                 tc.tile_pool(name="psA", bufs=3, space="PSUM") as psA, \
                 tc.tile_pool(name="psV", bufs=2, space="PSUM") as psV:
                w1t = []
                for k in range(KT):
                    t = w1p.tile([128, UNITS], bf16, tag=f"w1_{k}")
                    nc.sync.dma_start(t[:], W1[128 * k:128 * (k + 1), :])
                    w1t.append(t)
                et_full = []
                for n in range(NTK):
                    nsl = slice(512 * n, 512 * (n + 1))
                    ett = []
                    for k in range(KT):
                        t = etp.tile([128, 512], bf16, tag=f"et{k}")
                        nc.sync.dma_start(t[:], ET[128 * k:128 * (k + 1), nsl])
                        ett.append(t)
                    sts = []
                    for m in range(MT):
                        msl = slice(128 * m, 128 * (m + 1))
                        ps = psA.tile([128, 512], f32, tag="psA")
                        for k in range(KT):
                            nc.tensor.matmul(ps[:], w1t[k][:, msl], ett[k][:],
                                             start=(k == 0), stop=False)
                        nc.tensor.matmul(ps[:], hpp[:, msl], indt[:, nsl],
                                         start=False, stop=True)
                        st = stp.tile([128, 512], bf16, tag=f"st{m}")
                        nc.scalar.activation(st[:], ps[:], AF.Tanh)
                        sts.append(st)
                    pv = psV.tile([1, 512], f32, tag="psV")
                    for m in range(MT):
                        nc.tensor.matmul(pv[:], vt[m][:], sts[m][:],
                                         start=(m == 0), stop=(m == MT - 1))
                    nc.scalar.copy(sv[:, nsl], pv[:])
                    if n == 0:
                        # emit E loads after phase-A n=0 tiles so they don't
                        # starve the ET stream; they land during phase A
                        for k in range(TOK // 128):
                            t = ep.tile([128, ENC], bf16, tag=f"e{k}",
                                        name=f"e{k}")
                            nc.sync.dma_start(t[:], E[128 * k:128 * (k + 1), :])
                            et_full.append(t)

            # ---- softmax over L within each batch row
            # route [1,2048] -> [32,64] reshape through DRAM (unambiguous)
            scratch = nc.dram_tensor("SCRATCH", [BC, L], f32).ap()
            nc.sync.dma_start(scratch[:], sv[:])
            sc = pp.tile([BC, L], f32, tag="sc")
            nc.sync.dma_start(sc[:], scratch[:])
            if debug:
                nc.sync.dma_start(DSC[:], sc[:])
            negmax = pp.tile([BC, 1], f32, tag="negmax")
            nc.vector.tensor_reduce(negmax[:], sc[:], axis=mybir.AxisListType.X,
                                    op=mybir.AluOpType.max, negate=True)
            ex = pp.tile([BC, L], f32, tag="ex")
            sumex = pp.tile([BC, 1], f32, tag="sumex")
            nc.scalar.activation(ex[:], sc[:], AF.Exp, bias=negmax[:],
                                 scale=1.0, accum_out=sumex[:])
            rec = pp.tile([BC, 1], f32, tag="rec")
            nc.vector.reciprocal(rec[:], sumex[:])
            aw = pp.tile([BC, L], f32, tag="aw")
            nc.vector.tensor_scalar_mul(aw[:], ex[:], rec[:])
            nc.sync.dma_start(AW[:], aw[:])

            # ---- W_blk [tok-tiles, 32] from attention weights
            wtd = pp.tile([128, BC], f32, tag="wtd")   # wT duplicated over halves
            nc.vector.transpose(wtd[0:32, :], aw[:, 0:32])
            nc.vector.transpose(wtd[32:64, :], aw[:, 32:64])
            # duplicate rows 0:64 into 64:128 via DMA (cross-partition move)
            nc.sync.dma_start(wtd[64:128, :], wtd[0:64, :])
            wblk = pp.tile([128, 16 * BC], bf16, tag="wblk")
            nc.vector.memset(wblk[:], 0.0)
            nc.vector.tensor_copy(wblk[0:64, 0:512:34], wtd[0:64, 0:32:2])
            nc.vector.tensor_copy(wblk[64:128, 1:512:34], wtd[64:128, 1:32:2])

            # ---- context^T: for each enc tile accumulate over tok tiles
            with tc.tile_pool(name="psC", bufs=3, space="PSUM") as psC:
                for m in range(ENC // 128):
                    msl = slice(128 * m, 128 * (m + 1))
                    ps = psC.tile([128, BC], f32, tag="psC")
                    for k in range(TOK // 128):
                        nc.tensor.matmul(ps[:], et_full[k][:, msl],
                                         wblk[:, BC * k:BC * (k + 1)],
                                         start=(k == 0), stop=(k == TOK // 128 - 1))
                    nc.scalar.copy(xt[m][:], ps[:])
            if debug:
                for k in range(GIN // 128):
                    nc.sync.dma_start(DXT[128 * k:128 * (k + 1), :], xt[k][:])

            # ---- GRU: xm = x @ gru_k + bias-ext  (out [BC, 3U])
            zsb = pp.tile([BC, UNITS], f32, tag="zsb")
            rsb = pp.tile([BC, UNITS], f32, tag="rsb")
            xhsb = pp.tile([BC, UNITS], f32, tag="xhsb")
            NG = 3 * UNITS // 512      # 6 n-tiles
            with tc.tile_pool(name="gkp", bufs=10) as gkp, \
                 tc.tile_pool(name="psG", bufs=1, space="PSUM") as psG:
                pgs = [psG.tile([BC, 512], f32, tag=f"psG{n}", name=f"psG{n}")
                       for n in range(NG)]
                for k in range(GIN // 128):
                    gk = gkp.tile([128, 3 * UNITS], bf16, tag="gk")
                    nc.sync.dma_start(gk[:], GRUK[128 * k:128 * (k + 1), :])
                    for n in range(NG):
                        nc.tensor.matmul(pgs[n][:], xt[k][:],
                                         gk[:, 512 * n:512 * (n + 1)],
                                         start=(k == 0), stop=False)
                for n in range(NG):
                    nc.tensor.matmul(pgs[n][:], ones32[:],
                                     gbrow[:, 512 * n:512 * (n + 1)],
                                     start=False, stop=True)
                    osl = slice(512 * (n % 2), 512 * (n % 2 + 1))
                    if n < 2:
                        nc.scalar.activation(zsb[:, osl], pgs[n][:], AF.Sigmoid)
                    elif n < 4:
                        nc.scalar.activation(rsb[:, osl], pgs[n][:], AF.Sigmoid)
                    else:
                        nc.scalar.copy(xhsb[:, osl], pgs[n][:])
            ep_cm.__exit__(None, None, None)
            if debug:
                nc.sync.dma_start(DXM[:, 0:UNITS], zsb[:])
                nc.sync.dma_start(DXM[:, UNITS:2 * UNITS], rsb[:])
                nc.sync.dma_start(DXM[:, 2 * UNITS:3 * UNITS], xhsb[:])

            cand = pp.tile([BC, UNITS], f32, tag="cand")
            with tc.tile_pool(name="psHH", bufs=2, space="PSUM") as psHH:
                for h in range(2):
                    sl = slice(512 * h, 512 * (h + 1))
                    ph = psHH.tile([BC, 512], f32, tag="psHH")
                    nc.tensor.matmul(ph[:], ones32[:], hhrow[:, sl],
                                     start=True, stop=True)
                    # rh = r * hh ; cand_pre = xh + rh
                    rh = pp.tile([BC, 512], f32, tag=f"rh{h}")
                    nc.vector.tensor_tensor(rh[:], rsb[:, sl], ph[:],
                                            op=mybir.AluOpType.mult)
                    nc.vector.tensor_tensor(rh[:], xhsb[:, sl], rh[:],
                                            op=mybir.AluOpType.add)
                    nc.scalar.activation(cand[:, sl], rh[:], AF.Tanh)
            onem = pp.tile([BC, UNITS], f32, tag="onem")
            nc.scalar.activation(onem[:], zsb[:], AF.Copy, bias=1.0, scale=-1.0)
            fs = pp.tile([BC, UNITS], f32, tag="fs")
            nc.vector.tensor_tensor(fs[:], onem[:], cand[:],
                                    op=mybir.AluOpType.mult)
            nc.sync.dma_start(FS[:], fs[:])
            fsb = pp.tile([BC, UNITS], bf16, tag="fsb")
            nc.vector.tensor_copy(fsb[:], fs[:])

            # ---- fs^T via PE transpose, then o1^T = fc1^T fs^T + b
            with tc.tile_pool(name="f1p", bufs=1) as f1p, \
                 tc.tile_pool(name="pst", bufs=2, space="PSUM") as pst, \
                 tc.tile_pool(name="psO", bufs=2, space="PSUM") as psO:
                fst = []
                for j in range(MT):
                    ps = pst.tile([128, BC], bf16, tag="pst")
                    nc.tensor.transpose(ps[:], fsb[:, 128 * j:128 * (j + 1)],
                                        ident[0:BC, 0:BC])
                    t = pp.tile([128, BC], bf16, tag=f"fst{j}")
                    nc.scalar.copy(t[:], ps[:])
                    fst.append(t)
                f1t = []
                for k in range(MT):
                    t = f1p.tile([128, UNITS], bf16, tag=f"f1_{k}")
                    nc.sync.dma_start(t[:], FC1[128 * k:128 * (k + 1), :])
                    f1t.append(t)
                for m in range(MT):
                    msl = slice(128 * m, 128 * (m + 1))
                    ps = psO.tile([128, BC], f32, tag="psO")
                    for k in range(MT):
                        nc.tensor.matmul(ps[:], f1t[k][:, msl], fst[k][:],
                                         start=(k == 0), stop=(k == MT - 1))
                    ot = pp.tile([128, BC], bf16, tag=f"o1t{m}")
                    nc.scalar.activation(ot[:], ps[:], AF.Identity,
                                         bias=fc1b[:, m:m + 1])
                    nc.sync.dma_start(O1T[128 * m:128 * (m + 1), :], ot[:])
    nc.compile()
    return nc


# ----------------------------------------------------------------------------
# Launch 2: logits = o1 @ fc2_w + fc2_b, vocab-parallel.
# ----------------------------------------------------------------------------
def build_l2():
    nc = bacc.Bacc("TRN2", target_bir_lowering=False, debug=False,
                   enable_asserts=True, num_devices=NCORES)
    dt = nc.dram_tensor
    O1TF = dt("O1TF", [UNITS, B], bf16, kind="ExternalInput").ap()
    FC2 = dt("FC2", [UNITS, VS], bf16, kind="ExternalInput").ap()
    FC2B = dt("FC2B", [1, VS], bf16, kind="ExternalInput").ap()
    LG = dt("LG", [B, VS], f32, kind="ExternalOutput").ap()

    KT = UNITS // 128   # 8
    NV = VS // 512      # 13

    with tile.TileContext(nc) as tc:
        with tc.tile_pool(name="pp", bufs=1) as pp, \
             tc.tile_pool(name="fc2p", bufs=1) as fc2p, \
             tc.tile_pool(name="outp", bufs=3) as outp, \
             tc.tile_pool(name="ps", bufs=1, space="PSUM") as psp:
            ones = pp.tile([1, 128], bf16, tag="ones")
            nc.vector.memset(ones[:], 1.0)
            b2row = pp.tile([1, VS], bf16, tag="b2row")
            nc.sync.dma_start(b2row[:], FC2B[:])
            o1k = []
            for k in range(KT):
                t = pp.tile([128, B], bf16, tag=f"o1_{k}")
                nc.sync.dma_start(t[:], O1TF[128 * k:128 * (k + 1), :])
                o1k.append(t)
            f2t = []
            for k in range(KT):
                t = fc2p.tile([128, VS], bf16, tag=f"f2_{k}")
                nc.sync.dma_start(t[:], FC2[128 * k:128 * (k + 1), :])
                f2t.append(t)
            for h in range(2):
                hsl = slice(128 * h, 128 * (h + 1))
                for chunk in ((0, 7), (7, 13)):
                    pss = {}
                    for vt in range(*chunk):
                        pss[vt] = psp.tile([128, 512], f32, tag=f"ps{vt - chunk[0]}", name=f"ps{vt}")
                    for k in range(KT):
                        for vt in range(*chunk):
                            vsl = slice(512 * vt, 512 * (vt + 1))
                            nc.tensor.matmul(pss[vt][:], o1k[k][:, hsl],
                                             f2t[k][:, vsl],
                                             start=(k == 0), stop=False)
                    for vt in range(*chunk):
                        vsl = slice(512 * vt, 512 * (vt + 1))
                        nc.tensor.matmul(pss[vt][:], ones[:], b2row[:, vsl],
                                         start=False, stop=True)
                        so = outp.tile([128, 512], f32, tag="so")
                        nc.scalar.copy(so[:], pss[vt][:])
                        nc.sync.dma_start(LG[hsl, vsl], so[:])
    nc.compile()
    return nc


# ----------------------------------------------------------------------------
# Host-side prep, launches, gather.
# ----------------------------------------------------------------------------
_NC_CACHE = {}


def _get_nc(name):
    if name not in _NC_CACHE:
        _NC_CACHE[name] = {"l1": build_l1, "l2": build_l2}[name]()
    return _NC_CACHE[name]


def _bf(a):
    return np.ascontiguousarray(np.asarray(a, np.float32)).astype(BF)


def prep_l1_maps(X, encoderOut, hidden, emb, W1, b1, W2, b2, v,
                 gru_b, fc1_w, fc1_b):
    X = np.asarray(X).astype(np.int64).reshape(B)
    enc = np.asarray(encoderOut, np.float32)
    hid = np.asarray(hidden, np.float32)
    emb = np.asarray(emb, np.float32)
    W1b = _bf(W1)
    W2b = _bf(W2)
    vb = _bf(np.asarray(v, np.float32).reshape(UNITS, 1))
    b12 = _bf((np.asarray(b1, np.float32) + np.asarray(b2, np.float32))
              .reshape(1, UNITS))
    ind = _bf(np.kron(np.eye(BC, dtype=np.float32), np.ones((1, L), np.float32)))
    gb = np.asarray(gru_b, np.float32)
    gbrow = np.concatenate([gb[0, :2 * UNITS] + gb[1, :2 * UNITS],
                            gb[0, 2 * UNITS:]]).reshape(1, 3 * UNITS)
    gbrow = _bf(gbrow)
    hh = _bf(gb[1, 2 * UNITS:].reshape(1, UNITS))
    fc1b = np.ascontiguousarray(
        np.asarray(fc1_b, np.float32).reshape(UNITS // 128, 128).T)
    fc1wb = _bf(fc1_w)

    maps = []
    for c in range(NCORES):
        rows = slice(BC * c, BC * (c + 1))
        Ec = np.ascontiguousarray(enc[rows].reshape(TOK, ENC))
        maps.append(dict(
            ET=_bf(Ec.T), E=_bf(Ec), W1=W1b, W2=W2b,
            HIDT=_bf(hid[rows].T), B12=b12, IND=ind, V=vb,
            XET=_bf(emb[X[rows]].T),
            GBROW=gbrow, HH=hh, FC1=fc1wb, FC1B=fc1b,
        ))
    return maps


def kernel(X, encoderOut, hidden, emb, W1, b1, W2, b2, v, bv,
           gru_k, gru_rk, gru_b, fc1_w, fc1_b, fc2_w, fc2_b):
    # --- launch 1 ---
    maps1 = prep_l1_maps(X, encoderOut, hidden, emb, W1, b1, W2, b2, v,
                         gru_b, fc1_w, fc1_b)
    gkb = _bf(gru_k)
    for m in maps1:
        m["GRUK"] = gkb
    nc1 = _get_nc("l1")
    res1 = run_bass_kernel_spmd(nc1, maps1, core_ids=CORES).results

    # --- gather o1^T, prep launch 2 ---
    o1t_full = np.concatenate([np.asarray(r["O1T"]) for r in res1], axis=1)
    o1t_full = np.ascontiguousarray(o1t_full).astype(BF, copy=False)
    fc2 = np.zeros((UNITS, VPAD), np.float32)
    fc2[:, :VOCAB] = np.asarray(fc2_w, np.float32)
    fc2 = fc2.astype(BF)
    fc2b = np.zeros((1, VPAD), np.float32)
    fc2b[0, :VOCAB] = np.asarray(fc2_b, np.float32)
    fc2b = fc2b.astype(BF)
    maps2 = []
    for c in range(NCORES):
        csl = slice(VS * c, VS * (c + 1))
        maps2.append(dict(O1TF=o1t_full,
                          FC2=np.ascontiguousarray(fc2[:, csl]),
                          FC2B=np.ascontiguousarray(fc2b[:, csl])))
    nc2 = _get_nc("l2")
    res2 = run_bass_kernel_spmd(nc2, maps2, core_ids=CORES).results

    # --- gather outputs ---
    logits = np.concatenate([np.asarray(r["LG"], np.float32) for r in res2],
                            axis=1)[:, :VOCAB]
    finalState = np.concatenate([np.asarray(r["FS"], np.float32) for r in res1],
                                axis=0)
    attenWts = np.concatenate([np.asarray(r["AW"], np.float32) for r in res1],
                              axis=0).reshape(B, L, 1)
    return logits, finalState, attenWts


# revision 12
# speedup vs baseline: 1.3121x; 1.0248x over previous
"""Trainium2 Bass kernel for nn_Decoder_57758720196948.

Bahdanau-attention decoder step:
  score   = tanh(encoderOut @ W1 + b1 + (hidden @ W2 + b2)[:,None,:])
  attenW  = softmax(score @ v + bv, axis=L)          (bv drops out of softmax)
  context = sum(attenW * encoderOut, axis=L)
  x       = [context ; emb[X]]
  GRU with h_prev = 0  =>  hm = gru_b[1], gru_rk unused,
      z = sigmoid(xz+hz), r = sigmoid(xr+hr), cand = tanh(xh + r*hh),
      h' = (1-z)*cand
  o1      = h' @ fc1_w + fc1_b
  logits  = o1 @ fc2_w + fc2_b

Sharding: launch 1 is data-parallel over batch (32 rows/core); launch 2 is
tensor-parallel over the vocab axis of fc2 (6656 padded cols/core), with the
gather of logits done host-side. Matmuls run in bf16 with fp32 PSUM
accumulation; softmax/GRU elementwise math in fp32.
"""
import numpy as np
import ml_dtypes

import concourse.bass as bass
import concourse.tile as tile
from concourse import bacc, mybir
from concourse.bass_utils import run_bass_kernel_spmd
from concourse.masks import make_identity

bf16 = mybir.dt.bfloat16
f32 = mybir.dt.float32
AF = mybir.ActivationFunctionType
BF = ml_dtypes.bfloat16

B, L, ENC, UNITS, EMB, VOCAB = 256, 64, 2048, 1024, 512, 50257
NCORES = 8
BC = B // NCORES            # 32 batch rows per core
TOK = BC * L                # 2048 tokens per core
GIN = ENC + EMB             # 2560
VS = 6656                   # padded vocab shard (13 * 512)
VPAD = VS * NCORES          # 53248
CORES = list(range(NCORES))


# ----------------------------------------------------------------------------
# Launch 1: attention + GRU + fc1, data-parallel over batch.
# ----------------------------------------------------------------------------
def build_l1(debug: bool = False):
    nc = bacc.Bacc("TRN2", target_bir_lowering=False, debug=False,
                   enable_asserts=True, num_devices=NCORES)
    dt = nc.dram_tensor
    ET = dt("ET", [ENC, TOK], bf16, kind="ExternalInput").ap()      # E^T (enc, tok)
    E = dt("E", [TOK, ENC], bf16, kind="ExternalInput").ap()        # E (tok, enc)
    W1 = dt("W1", [ENC, UNITS], bf16, kind="ExternalInput").ap()
    W2 = dt("W2", [UNITS, UNITS], bf16, kind="ExternalInput").ap()
    HIDT = dt("HIDT", [UNITS, BC], bf16, kind="ExternalInput").ap()  # hidden^T
    B12 = dt("B12", [1, UNITS], bf16, kind="ExternalInput").ap()     # b1+b2
    IND = dt("IND", [BC, TOK], bf16, kind="ExternalInput").ap()      # IND[b,t]=(t//L==b)
    V = dt("V", [UNITS, 1], bf16, kind="ExternalInput").ap()
    XET = dt("XET", [EMB, BC], bf16, kind="ExternalInput").ap()      # emb[X]^T
    GRUK = dt("GRUK", [GIN, 3 * UNITS], bf16, kind="ExternalInput").ap()
    GBROW = dt("GBROW", [1, 3 * UNITS], bf16, kind="ExternalInput").ap()  # folded gru bias
    HH = dt("HH", [1, UNITS], bf16, kind="ExternalInput").ap()       # gru_b[1][2U:3U]
    FC1 = dt("FC1", [UNITS, UNITS], bf16, kind="ExternalInput").ap()
    FC1B = dt("FC1B", [128, UNITS // 128], f32, kind="ExternalInput").ap()

    AW = dt("AW", [BC, L], f32, kind="ExternalOutput").ap()
    FS = dt("FS", [BC, UNITS], f32, kind="ExternalOutput").ap()
    O1T = dt("O1T", [UNITS, BC], bf16, kind="ExternalOutput").ap()
    if debug:
        DSC = dt("DSC", [BC, L], f32, kind="ExternalOutput").ap()       # raw score
        DHPP = dt("DHPP", [BC, UNITS], bf16, kind="ExternalOutput").ap()
        DXT = dt("DXT", [GIN, BC], bf16, kind="ExternalOutput").ap()    # x^T
        DXM = dt("DXM", [BC, 3 * UNITS], f32, kind="ExternalOutput").ap()

    KT = ENC // 128      # 16 k-tiles over enc
    MT = UNITS // 128    # 8 m-tiles over units
    NTK = TOK // 512     # 4 n-tiles over tokens

    with tile.TileContext(nc) as tc:
        with tc.tile_pool(name="persist", bufs=1) as pp:
            # ---- persistent small tiles
            ones32 = pp.tile([1, BC], bf16, tag="ones32")
            nc.vector.memset(ones32[:], 1.0)
            b12 = pp.tile([1, UNITS], bf16, tag="b12")
            nc.sync.dma_start(b12[:], B12[:])
            hid = []
            for k in range(MT):
                t = pp.tile([128, BC], bf16, tag=f"hid{k}")
                nc.sync.dma_start(t[:], HIDT[128 * k:128 * (k + 1), :])
                hid.append(t)
            ident = pp.tile([128, 128], bf16, tag="ident")
            make_identity(nc, ident)
            indt = pp.tile([BC, TOK], bf16, tag="ind")
            nc.sync.dma_start(indt[:], IND[:])
            vt = []
            for m in range(MT):
                t = pp.tile([128, 1], bf16, tag=f"v{m}")
                nc.sync.dma_start(t[:], V[128 * m:128 * (m + 1), :])
                vt.append(t)
            hpp = pp.tile([BC, UNITS], bf16, tag="hpp")
            gbrow = pp.tile([1, 3 * UNITS], bf16, tag="gbrow")
            nc.sync.dma_start(gbrow[:], GBROW[:])
            hhrow = pp.tile([1, UNITS], bf16, tag="hh")
            nc.sync.dma_start(hhrow[:], HH[:])
            fc1b = pp.tile([128, UNITS // 128], f32, tag="fc1b")
            nc.sync.dma_start(fc1b[:], FC1B[:])
            # x^T tiles: 16 context + 4 embedding
            xt = []
            for k in range(GIN // 128):
                xt.append(pp.tile([128, BC], bf16, tag=f"xt{k}", name=f"xt{k}"))
            for j in range(EMB // 128):
                nc.sync.dma_start(xt[KT + j][:], XET[128 * j:128 * (j + 1), :])

            # ---- H'' = hidden @ W2 + (b1+b2)   -> hpp [BC, UNITS] bf16
            with tc.tile_pool(name="w2p", bufs=1) as w2p, \
                 tc.tile_pool(name="psh", bufs=2, space="PSUM") as psh:
                w2t = []
                for k in range(MT):
                    t = w2p.tile([128, UNITS], bf16, tag=f"w2_{k}")
                    nc.sync.dma_start(t[:], W2[128 * k:128 * (k + 1), :])
                    w2t.append(t)
                for h in range(2):
                    ps = psh.tile([BC, 512], f32, tag="psh")
                    sl = slice(512 * h, 512 * (h + 1))
                    for k in range(MT):
                        nc.tensor.matmul(ps[:], hid[k][:], w2t[k][:, sl],
                                         start=(k == 0), stop=False)
                    nc.tensor.matmul(ps[:], ones32[:], b12[:, sl],
                                     start=False, stop=True)
                    nc.scalar.copy(hpp[:, sl], ps[:])
            if debug:
                nc.sync.dma_start(DHPP[:], hpp[:])

            # ---- phase A: S^T = tanh(W1^T ET + H''-ext), score = v^T S^T
            scratch = nc.dram_tensor("SCRATCH", [BC, L], f32).ap()
            sv = pp.tile([1, TOK], f32, tag="sv")
            ep = tc.alloc_tile_pool(name="ep", bufs=1)
            with tc.tile_pool(name="w1p", bufs=1) as w1p, \
                 tc.tile_pool(name="etp", bufs=2) as etp, \
                 tc.tile_pool(name="stp", bufs=2) as stp, \
                 tc.tile_pool(name="psA", bufs=3, space="PSUM") as psA, \
                 tc.tile_pool(name="psV", bufs=2, space="PSUM") as psV:
                w1t = []
                for k in range(KT):
                    t = w1p.tile([128, UNITS], bf16, tag=f"w1_{k}")
                    nc.sync.dma_start(t[:], W1[128 * k:128 * (k + 1), :])
                    w1t.append(t)
                et_full = []
                for n in range(NTK):
                    nsl = slice(512 * n, 512 * (n + 1))
                    ett = []
                    for k in range(KT):
                        t = etp.tile([128, 512], bf16, tag=f"et{k}")
                        nc.sync.dma_start(t[:], ET[128 * k:128 * (k + 1), nsl])
                        ett.append(t)
                    sts = []
                    for m in range(MT):
                        msl = slice(128 * m, 128 * (m + 1))
                        ps = psA.tile([128, 512], f32, tag="psA")
                        for k in range(KT):
                            nc.tensor.matmul(ps[:], w1t[k][:, msl], ett[k][:],
                                             start=(k == 0), stop=False)
                        nc.tensor.matmul(ps[:], hpp[:, msl], indt[:, nsl],
                                         start=False, stop=True)
                        st = stp.tile([128, 512], bf16, tag=f"st{m}")
                        nc.scalar.activation(st[:], ps[:], AF.Tanh)
                        sts.append(st)
                    pv = psV.tile([1, 512], f32, tag="psV")
                    for m in range(MT):
                        nc.tensor.matmul(pv[:], vt[m][:], sts[m][:],
                                         start=(m == 0), stop=(m == MT - 1))
                    nc.scalar.copy(sv[:, nsl], pv[:])
                    nc.sync.dma_start(scratch[8 * n:8 * (n + 1), :], sv[:, nsl])
                # E loads land during the tail of phase A (after all ET tiles)
                for k in range(TOK // 128):
                    t = ep.tile([128, ENC], bf16, tag=f"e{k}", name=f"e{k}")
                    nc.sync.dma_start(t[:], E[128 * k:128 * (k + 1), :])
                    et_full.append(t)

            # ---- softmax over L within each batch row
            sc = pp.tile([BC, L], f32, tag="sc")
            nc.sync.dma_start(sc[:], scratch[:])
            if debug:
                nc.sync.dma_start(DSC[:], sc[:])
            negmax = pp.tile([BC, 1], f32, tag="negmax")
            nc.vector.tensor_reduce(negmax[:], sc[:], axis=mybir.AxisListType.X,
                                    op=mybir.AluOpType.max, negate=True)
            ex = pp.tile([BC, L], f32, tag="ex")
            sumex = pp.tile([BC, 1], f32, tag="sumex")
            nc.scalar.activation(ex[:], sc[:], AF.Exp, bias=negmax[:],
                                 scale=1.0, accum_out=sumex[:])
            rec = pp.tile([BC, 1], f32, tag="rec")
            nc.vector.reciprocal(rec[:], sumex[:])
            aw = pp.tile([BC, L], f32, tag="aw")
            nc.vector.tensor_scalar_mul(aw[:], ex[:], rec[:])
            nc.sync.dma_start(AW[:], aw[:])

            # ---- W_blk [tok-tiles, 32] from attention weights
            wtd = pp.tile([128, BC], f32, tag="wtd")   # wT duplicated over halves
            nc.vector.transpose(wtd[0:32, :], aw[:, 0:32])
            nc.vector.transpose(wtd[32:64, :], aw[:, 32:64])
            # duplicate rows 0:64 into 64:128 via DMA (cross-partition move)
            nc.sync.dma_start(wtd[64:128, :], wtd[0:64, :])
            wblk = pp.tile([128, 16 * BC], bf16, tag="wblk")
            nc.vector.memset(wblk[:], 0.0)
            nc.vector.tensor_copy(wblk[0:64, 0:512:34], wtd[0:64, 0:32:2])
            nc.vector.tensor_copy(wblk[64:128, 1:512:34], wtd[64:128, 1:32:2])

            # ---- context^T: for each enc tile accumulate over tok tiles
            with tc.tile_pool(name="psC", bufs=3, space="PSUM") as psC:
                for m in range(ENC // 128):
                    msl = slice(128 * m, 128 * (m + 1))
                    ps = psC.tile([128, BC], f32, tag="psC")
                    for k in range(TOK // 128):
                        nc.tensor.matmul(ps[:], et_full[k][:, msl],
                                         wblk[:, BC * k:BC * (k + 1)],
                                         start=(k == 0), stop=(k == TOK // 128 - 1))
                    nc.scalar.copy(xt[m][:], ps[:])
            if debug:
                for k in range(GIN // 128):
                    nc.sync.dma_start(DXT[128 * k:128 * (k + 1), :], xt[k][:])

            # ---- GRU: xm = x @ gru_k + bias-ext  (out [BC, 3U])
            zsb = pp.tile([BC, UNITS], f32, tag="zsb")
            rsb = pp.tile([BC, UNITS], f32, tag="rsb")
            xhsb = pp.tile([BC, UNITS], f32, tag="xhsb")
            f1p = tc.alloc_tile_pool(name="f1p", bufs=1)
            f1t = []
            for k in range(MT):
                t = f1p.tile([128, UNITS], bf16, tag=f"f1_{k}", name=f"f1_{k}")
                nc.sync.dma_start(t[:], FC1[128 * k:128 * (k + 1), :])
                f1t.append(t)
            NG = 3 * UNITS // 512      # 6 n-tiles
            with tc.tile_pool(name="gkp", bufs=10) as gkp, \
                 tc.tile_pool(name="psG", bufs=1, space="PSUM") as psG:
                pgs = [psG.tile([BC, 512], f32, tag=f"psG{n}", name=f"psG{n}")
                       for n in range(NG)]
                for k in range(GIN // 128):
                    gk = gkp.tile([128, 3 * UNITS], bf16, tag="gk")
                    nc.sync.dma_start(gk[:], GRUK[128 * k:128 * (k + 1), :])
                    for n in range(NG):
                        nc.tensor.matmul(pgs[n][:], xt[k][:],
                                         gk[:, 512 * n:512 * (n + 1)],
                                         start=(k == 0), stop=False)
                for n in range(NG):
                    nc.tensor.matmul(pgs[n][:], ones32[:],
                                     gbrow[:, 512 * n:512 * (n + 1)],
                                     start=False, stop=True)
                    osl = slice(512 * (n % 2), 512 * (n % 2 + 1))
                    if n < 2:
                        nc.scalar.activation(zsb[:, osl], pgs[n][:], AF.Sigmoid)
                    elif n < 4:
                        nc.scalar.activation(rsb[:, osl], pgs[n][:], AF.Sigmoid)
                    else:
                        nc.scalar.copy(xhsb[:, osl], pgs[n][:])
            if debug:
                nc.sync.dma_start(DXM[:, 0:UNITS], zsb[:])
                nc.sync.dma_start(DXM[:, UNITS:2 * UNITS], rsb[:])
                nc.sync.dma_start(DXM[:, 2 * UNITS:3 * UNITS], xhsb[:])

            cand = pp.tile([BC, UNITS], f32, tag="cand")
            with tc.tile_pool(name="psHH", bufs=2, space="PSUM") as psHH:
                for h in range(2):
                    sl = slice(512 * h, 512 * (h + 1))
                    ph = psHH.tile([BC, 512], f32, tag="psHH")
                    nc.tensor.matmul(ph[:], ones32[:], hhrow[:, sl],
                                     start=True, stop=True)
                    # rh = r * hh ; cand_pre = xh + rh
                    rh = pp.tile([BC, 512], f32, tag=f"rh{h}")
                    nc.vector.tensor_tensor(rh[:], rsb[:, sl], ph[:],
                                            op=mybir.AluOpType.mult)
                    nc.vector.tensor_tensor(rh[:], xhsb[:, sl], rh[:],
                                            op=mybir.AluOpType.add)
                    nc.scalar.activation(cand[:, sl], rh[:], AF.Tanh)
            onem = pp.tile([BC, UNITS], f32, tag="onem")
            nc.scalar.activation(onem[:], zsb[:], AF.Copy, bias=1.0, scale=-1.0)
            fs = pp.tile([BC, UNITS], f32, tag="fs")
            nc.vector.tensor_tensor(fs[:], onem[:], cand[:],
                                    op=mybir.AluOpType.mult)
            nc.sync.dma_start(FS[:], fs[:])
            fsb = pp.tile([BC, UNITS], bf16, tag="fsb")
            nc.vector.tensor_copy(fsb[:], fs[:])

            # ---- fs^T via PE transpose, then o1^T = fc1^T fs^T + b
            with tc.tile_pool(name="pst", bufs=2, space="PSUM") as pst, \
                 tc.tile_pool(name="psO", bufs=2, space="PSUM") as psO:
                fst = []
                for j in range(MT):
                    ps = pst.tile([128, BC], bf16, tag="pst")
                    nc.tensor.transpose(ps[:], fsb[:, 128 * j:128 * (j + 1)],
                                        ident[0:BC, 0:BC])
                    t = pp.tile([128, BC], bf16, tag=f"fst{j}")
                    nc.scalar.copy(t[:], ps[:])
                    fst.append(t)
                for m in range(MT):
                    msl = slice(128 * m, 128 * (m + 1))
                    ps = psO.tile([128, BC], f32, tag="psO")
                    for k in range(MT):
                        nc.tensor.matmul(ps[:], f1t[k][:, msl], fst[k][:],
                                         start=(k == 0), stop=(k == MT - 1))
                    ot = pp.tile([128, BC], bf16, tag=f"o1t{m}")
                    nc.scalar.activation(ot[:], ps[:], AF.Identity,
                                         bias=fc1b[:, m:m + 1])
                    nc.sync.dma_start(O1T[128 * m:128 * (m + 1), :], ot[:])
            f1p.release()
            ep.release()
    nc.compile()
    return nc


# ----------------------------------------------------------------------------
# Launch 2: logits = o1 @ fc2_w + fc2_b, vocab-parallel.
# ----------------------------------------------------------------------------
def build_l2():
    nc = bacc.Bacc("TRN2", target_bir_lowering=False, debug=False,
                   enable_asserts=True, num_devices=NCORES)
    dt = nc.dram_tensor
    O1TF = dt("O1TF", [UNITS, B], bf16, kind="ExternalInput").ap()
    FC2 = dt("FC2", [UNITS, VS], bf16, kind="ExternalInput").ap()
    FC2B = dt("FC2B", [1, VS], bf16, kind="ExternalInput").ap()
    LG = dt("LG", [B, VS], f32, kind="ExternalOutput").ap()

    KT = UNITS // 128   # 8
    NV = VS // 512      # 13

    with tile.TileContext(nc) as tc:
        with tc.tile_pool(name="pp", bufs=1) as pp, \
             tc.tile_pool(name="fc2p", bufs=1) as fc2p, \
             tc.tile_pool(name="outp", bufs=6) as outp, \
             tc.tile_pool(name="ps", bufs=1, space="PSUM") as psp:
            ones = pp.tile([1, 128], bf16, tag="ones")
            nc.vector.memset(ones[:], 1.0)
            b2row = pp.tile([1, VS], bf16, tag="b2row")
            nc.sync.dma_start(b2row[:], FC2B[:])
            o1k = []
            for k in range(KT):
                t = pp.tile([128, B], bf16, tag=f"o1_{k}")
                nc.sync.dma_start(t[:], O1TF[128 * k:128 * (k + 1), :])
                o1k.append(t)
            f2t = []
            for k in range(KT):
                f2t.append(fc2p.tile([128, VS], bf16, tag=f"f2_{k}",
                                     name=f"f2_{k}"))
            for c0, c1 in ((0, 3584), (3584, VS)):
                for k in range(KT):
                    nc.sync.dma_start(f2t[k][:, c0:c1],
                                      FC2[128 * k:128 * (k + 1), c0:c1])
            for h in range(2):
                hsl = slice(128 * h, 128 * (h + 1))
                for chunk in ((0, 7), (7, 13)):
                    pss = {}
                    for vt in range(*chunk):
                        pss[vt] = psp.tile([128, 512], f32, tag=f"ps{vt - chunk[0]}", name=f"ps{vt}")
                    for k in range(KT):
                        for vt in range(*chunk):
                            vsl = slice(512 * vt, 512 * (vt + 1))
                            nc.tensor.matmul(pss[vt][:], o1k[k][:, hsl],
                                             f2t[k][:, vsl],
                                             start=(k == 0), stop=False)
                    for vt in range(*chunk):
                        vsl = slice(512 * vt, 512 * (vt + 1))
                        nc.tensor.matmul(pss[vt][:], ones[:], b2row[:, vsl],
                                         start=False, stop=True)
                        so = outp.tile([128, 512], f32, tag="so")
                        nc.scalar.copy(so[:], pss[vt][:])
                        nc.sync.dma_start(LG[hsl, vsl], so[:])
    nc.compile()
    return nc


# ----------------------------------------------------------------------------
# Host-side prep, launches, gather.
# ----------------------------------------------------------------------------
_NC_CACHE = {}


def _get_nc(name):
    if name not in _NC_CACHE:
        _NC_CACHE[name] = {"l1": build_l1, "l2": build_l2}[name]()
    return _NC_CACHE[name]


def _bf(a):
    return np.ascontiguousarray(np.asarray(a, np.float32)).astype(BF)


def prep_l1_maps(X, encoderOut, hidden, emb, W1, b1, W2, b2, v,
                 gru_b, fc1_w, fc1_b):
    X = np.asarray(X).astype(np.int64).reshape(B)
    enc = np.asarray(encoderOut, np.float32)
    hid = np.asarray(hidden, np.float32)
    emb = np.asarray(emb, np.float32)
    W1b = _bf(W1)
    W2b = _bf(W2)
    vb = _bf(np.asarray(v, np.float32).reshape(UNITS, 1))
    b12 = _bf((np.asarray(b1, np.float32) + np.asarray(b2, np.float32))
              .reshape(1, UNITS))
    ind = _bf(np.kron(np.eye(BC, dtype=np.float32), np.ones((1, L), np.float32)))
    gb = np.asarray(gru_b, np.float32)
    gbrow = np.concatenate([gb[0, :2 * UNITS] + gb[1, :2 * UNITS],
                            gb[0, 2 * UNITS:]]).reshape(1, 3 * UNITS)
    gbrow = _bf(gbrow)
    hh = _bf(gb[1, 2 * UNITS:].reshape(1, UNITS))
    fc1b = np.ascontiguousarray(
        np.asarray(fc1_b, np.float32).reshape(UNITS // 128, 128).T)
    fc1wb = _bf(fc1_w)

    maps = []
    for c in range(NCORES):
        rows = slice(BC * c, BC * (c + 1))
        Ec = np.ascontiguousarray(enc[rows].reshape(TOK, ENC))
        maps.append(dict(
            ET=_bf(Ec.T), E=_bf(Ec), W1=W1b, W2=W2b,
            HIDT=_bf(hid[rows].T), B12=b12, IND=ind, V=vb,
            XET=_bf(emb[X[rows]].T),
            GBROW=gbrow, HH=hh, FC1=fc1wb, FC1B=fc1b,
        ))
    return maps


def kernel(X, encoderOut, hidden, emb, W1, b1, W2, b2, v, bv,
           gru_k, gru_rk, gru_b, fc1_w, fc1_b, fc2_w, fc2_b):
    # --- launch 1 ---
    maps1 = prep_l1_maps(X, encoderOut, hidden, emb, W1, b1, W2, b2, v,
                         gru_b, fc1_w, fc1_b)
    gkb = _bf(gru_k)
    for m in maps1:
        m["GRUK"] = gkb
    nc1 = _get_nc("l1")
    res1 = run_bass_kernel_spmd(nc1, maps1, core_ids=CORES).results

    # --- gather o1^T, prep launch 2 ---
    o1t_full = np.concatenate([np.asarray(r["O1T"]) for r in res1], axis=1)
    o1t_full = np.ascontiguousarray(o1t_full).astype(BF, copy=False)
    fc2 = np.zeros((UNITS, VPAD), np.float32)
    fc2[:, :VOCAB] = np.asarray(fc2_w, np.float32)
    fc2 = fc2.astype(BF)
    fc2b = np.zeros((1, VPAD), np.float32)
    fc2b[0, :VOCAB] = np.asarray(fc2_b, np.float32)
    fc2b = fc2b.astype(BF)
    maps2 = []
    for c in range(NCORES):
        csl = slice(VS * c, VS * (c + 1))
        maps2.append(dict(O1TF=o1t_full,
                          FC2=np.ascontiguousarray(fc2[:, csl]),
                          FC2B=np.ascontiguousarray(fc2b[:, csl])))
    nc2 = _get_nc("l2")
    res2 = run_bass_kernel_spmd(nc2, maps2, core_ids=CORES).results

    # --- gather outputs ---
    logits = np.concatenate([np.asarray(r["LG"], np.float32) for r in res2],
                            axis=1)[:, :VOCAB]
    finalState = np.concatenate([np.asarray(r["FS"], np.float32) for r in res1],
                                axis=0)
    attenWts = np.concatenate([np.asarray(r["AW"], np.float32) for r in res1],
                              axis=0).reshape(B, L, 1)
    return logits, finalState, attenWts
